# revision 3
# baseline (speedup 1.0000x reference)
"""BGConv (GNN message passing) Trainium2 kernel, v2.

Design (node-sharded, 16 half-shards = 2 per core, no collectives):
  * Host routes every (edge, endpoint) contribution to the half-shard owning
    its destination node.  Each half-shard processes the deduplicated set of
    incident edges (<= ~25k < 2^15, so all device-side gather indices fit the
    int16 dma_gather format).
  * P1 (per half): edge endpoint features are dma_gather'ed (transposed,
    fp8) straight out of an SBUF-resident fp8 copy of object_feats; a
    DoubleRowSwInterleave fp8 matmul computes h = relu(x_sub@W1a + x_obj@W1b)
    per edge (edges on PSUM partitions), cast to fp8 and stored to a DRAM
    H table, p-major.  Edges are class-sorted (sub/obj endpoint < 32768 or
    not) so int16 indices address lo/hi halves of the node table.
  * P2 (per 128-node window): one dma_gather pulls all contribution H rows
    (fixed per-window chunk budget, idx-0 padded); host-precomputed fp8
    one-hot matrices (scaled w = 8192*exp(conf-10)) scatter-reduce them via
    matmuls into S_sub/S_obj PSUM accumulators.
  * P3 (per window): S is copied+transposed on PE, multiplied by the stacked
    W2 (bf16), then (numer + 8192*x) * recip/8192 with host-precomputed
    reciprocal denominators; contiguous DMA to the output shard.
  * Softmax bookkeeping (segment max == CONST, weights, denominators) is
    computed on host in f32 (asserted: confidence.max() < CONST-1).

SwInterleave note: the PE reverses stationary columns, so the h row of the
edge at stream slot s lands on PSUM partition 127-(s%128); the host accounts
for this in the H-row indices it hands to P2.
"""

import math
import numpy as np
import ml_dtypes

import concourse.bass as bass
import concourse.tile as tile
from concourse import bacc, mybir
from concourse.bass_utils import run_bass_kernel_spmd

# ---------------------------------------------------------------- constants
O_NODES = 50000
N_EDGES = 200000
D = 256
HIDDEN = 512
CONST = 10.0
N_CORES = 8
N_HALF = 16
HSHARD = O_NODES // N_HALF          # 3125
HPAD = 3200                         # padded nodes per half (25 windows)
NWIN = HPAD // 128                  # 25 windows per half
P = 128
SEG = 2048                          # max edges per P1 gather segment
WSCALE = 8192.0
LOSPLIT = 32768
F8 = np.dtype(mybir.dt.np(mybir.dt.float8e4))
NODE_RANKS = (O_NODES + P - 1) // P + 1      # 391 ranks of 128 nodes (padded)
NODE_PAD = NODE_RANKS * P

_BUILD_CACHE = {}


def _pack_idxs(idxs, n_slots):
    """[N] ints -> [128, n_slots//16] int16 wrapped (i at [i%16, i//16]),
    replicated across the 8 gpsimd partition groups."""
    t = np.zeros((16, n_slots // 16), dtype=np.int16)
    flat = np.asarray(idxs, dtype=np.int64)
    pos = np.arange(len(flat))
    t[pos % 16, pos // 16] = flat.astype(np.int16)
    return np.tile(t, (8, 1))


# ================================================================ host side
def _preprocess(object_feats, pairs, confidence, W1, b1, W2, b2):
    object_feats = np.asarray(object_feats, dtype=np.float32)
    pairs = np.asarray(pairs)
    confidence = np.asarray(confidence, dtype=np.float32)
    W1 = np.asarray(W1, dtype=np.float32)
    b1 = np.asarray(b1, dtype=np.float32)
    W2 = np.asarray(W2, dtype=np.float32)
    b2 = np.asarray(b2, dtype=np.float32)
    R = pairs.shape[0]

    conf_max = float(confidence.max())
    assert conf_max < CONST - 1.0, conf_max
    assert not np.any(b1), "b1 != 0 unsupported by this build"
    w_edge = np.exp(confidence - CONST)                    # (R,) f32

    sub = pairs[:, 0].astype(np.int64)
    obj = pairs[:, 1].astype(np.int64)

    # ---- per-half incident edge sets, class-sorted --------------------
    # Half h owns global nodes {n : n % 16 == h} (local index n // 16); the
    # stride-16 interleave decorrelates ownership from the lo/hi index split
    # so the per-class budgets are balanced across halves.
    halves = []
    for h in range(N_HALF):
        m = ((sub % N_HALF) == h) | ((obj % N_HALF) == h)
        eids = np.nonzero(m)[0]
        cls = (sub[eids] >= LOSPLIT) * 2 + (obj[eids] >= LOSPLIT)
        order = np.argsort(cls, kind="stable")
        eids = eids[order]
        cls = cls[order]
        cnt = np.bincount(cls, minlength=4)
        halves.append((eids, cnt))

    cb = np.zeros(4, dtype=np.int64)        # class budgets (multiple of 128)
    for _, cnt in halves:
        cb = np.maximum(cb, cnt)
    cb = (cb + P - 1) // P * P
    NT = int(cb.sum()) // P                 # tiles per half
    assert NT * P < LOSPLIT

    # P1 gather segment grid: per class run, segments of <= SEG edges
    seg_grid = []                           # (col_start, n_edges, class)
    base = 0
    for c in range(4):
        run = int(cb[c])
        off = 0
        while off < run:
            n = min(SEG, run - off)
            seg_grid.append((base + off, n, c))
            off += n
        base += run
    n_tiles_of = [(s[1] // P) for s in seg_grid]

    # ---- contributions routed to windows ------------------------------
    # contribution k: (dest node, edge, is_obj, weight)
    dest_all = np.concatenate([sub, obj])
    conf2 = np.concatenate([w_edge, w_edge])
    is_obj = np.concatenate([np.zeros(R, np.int8), np.ones(R, np.int8)])
    edge_all = np.concatenate([np.arange(R), np.arange(R)])

    # per (half, window): sub-count / obj-count for budget calc
    half_id = dest_all % N_HALF
    local_id = dest_all // N_HALF
    win_id = local_id // P
    CS = CO = 0
    for h in range(N_HALF):
        hm = half_id == h
        for half_type in (0, 1):
            tm = hm & (is_obj == half_type)
            c = np.bincount(win_id[tm], minlength=NWIN).max()
            if half_type == 0:
                CS = max(CS, int(c))
            else:
                CO = max(CO, int(c))
    CS = (CS + P - 1) // P * P // P        # chunks
    CO = (CO + P - 1) // P * P // P
    NCH = CS + CO
    NSLOT = NCH * P

    # ---- shared tensors ----------------------------------------------
    objq = np.zeros((NODE_PAD, D), dtype=F8)
    objq[:O_NODES] = object_feats.astype(F8)
    objb_pack = objq.reshape(NODE_RANKS, P, D).transpose(1, 0, 2).reshape(P, -1)
    w1_sub = W1[:D].astype(F8).reshape(P, 2, HIDDEN).reshape(P, -1)
    w1_obj = W1[D:].astype(F8).reshape(P, 2, HIDDEN).reshape(P, -1)
    # W2 stacked [1024, 256]: rows 0-511 -> W2[:, :256]; 512-1023 -> W2[:, 256:]
    w2s = np.concatenate([W2[:, :D], W2[:, D:]], axis=0)
    w2_pack = (
        w2s.reshape(8, P, D).transpose(1, 0, 2).reshape(P, -1)
        .astype(ml_dtypes.bfloat16)
    )
    ident = np.eye(P, dtype=np.float32).astype(ml_dtypes.bfloat16)

    # xw = x + dsub*b2a + dobj*b2b (b2 generic), prescaled by WSCALE
    dsub = np.bincount(sub, weights=w_edge, minlength=O_NODES)
    dobj = np.bincount(obj, weights=w_edge, minlength=O_NODES)
    xw = object_feats + np.outer(dsub, b2[:D]) + np.outer(dobj, b2[D:])
    xw *= WSCALE
    denom = 1.0 + dsub + dobj
    recip = (1.0 / (denom * WSCALE)).astype(np.float32)

    # ---- per-core tensors --------------------------------------------
    in_maps = []
    for c in range(N_CORES):
        p1_sub = np.zeros(2 * NT * P, dtype=np.int64)
        p1_obj = np.zeros(2 * NT * P, dtype=np.int64)
        p2_idx = np.zeros(2 * NWIN * NSLOT, dtype=np.int64)
        onehot = np.zeros((2 * NWIN * NCH * P, P), dtype=np.float32)
        xw_t = np.zeros((2 * HPAD, D), dtype=np.float32)
        rc_t = np.ones((P, 2 * NWIN), dtype=np.float32) / WSCALE

        # edge slot -> H row, accounting for SwInterleave column reversal
        def hrow(slot):
            return (127 - slot % P) * NT + slot // P

        for s in range(2):
            h = 2 * c + s
            eids, cnt = halves[h]
            # place class runs at budgeted offsets
            slot_of = np.full(R, -1, dtype=np.int64)   # edge -> stream slot
            base = 0
            epos = 0
            for cl in range(4):
                n = int(cnt[cl])
                ee = eids[epos:epos + n]
                slots = base + np.arange(n)
                p1_sub[s * NT * P + slots] = sub[ee] - (cl >= 2) * LOSPLIT
                p1_obj[s * NT * P + slots] = obj[ee] - (cl % 2) * LOSPLIT
                slot_of[ee] = slots
                epos += n
                base += int(cb[cl])

            # contributions of this half
            cm = half_id == h
            dl = local_id[cm]
            wl = conf2[cm] * WSCALE
            io = is_obj[cm]
            el = edge_all[cm]
            wi = dl // P
            for w in range(NWIN):
                for ht in (0, 1):
                    sel = (wi == w) & (io == ht)
                    k = int(sel.sum())
                    assert k <= (CS, CO)[ht] * P, (h, w, ht, k)
                    slot0 = (s * NWIN + w) * NSLOT + ht * CS * P
                    sl = slot_of[el[sel]]
                    assert (sl >= 0).all()
                    p2_idx[slot0:slot0 + k] = hrow(sl) + s * NT * P * 0
                    onehot[slot0:slot0 + k, :] = 0.0
                    onehot[slot0 + np.arange(k), dl[sel] - w * P] = wl[sel]
            xw_t[s * HPAD:s * HPAD + HSHARD] = xw[h::N_HALF]
            rec_h = recip[h::N_HALF]
            nwl = np.minimum(HSHARD - np.arange(NWIN) * P, P)
            for w in range(NWIN):
                rc_t[:nwl[w], s * NWIN + w] = rec_h[w * P: w * P + nwl[w]]

        oh_pack = (
            onehot.reshape(2 * NWIN * NCH, P, P).transpose(1, 0, 2)
            .reshape(P, -1).astype(F8)
        )
        in_maps.append({
            "objb": objb_pack, "w1s": w1_sub, "w1o": w1_obj, "w2": w2_pack,
            "ident": ident,
            "p1si": _pack_idxs(p1_sub, 2 * NT * P),
            "p1oi": _pack_idxs(p1_obj, 2 * NT * P),
            "p2i": _pack_idxs(p2_idx, 2 * NWIN * NSLOT),
            "oh": oh_pack, "xw": xw_t, "rc": rc_t,
        })

    dims = (NT, CS, CO, tuple(seg_grid))
    return in_maps, dims


# ================================================================ device side
def _build_program(dims):
    NT, CS, CO, seg_grid = dims
    NCH = CS + CO
    NSLOT = NCH * P
    dt = mybir.dt
    nc = bacc.Bacc("TRN2", target_bir_lowering=False, debug=False,
                   num_devices=N_CORES)

    objb_d = nc.dram_tensor("objb", [P, NODE_RANKS * D], dt.float8e4,
                            kind="ExternalInput").ap()
    w1s_d = nc.dram_tensor("w1s", [P, 2 * HIDDEN], dt.float8e4,
                           kind="ExternalInput").ap()
    w1o_d = nc.dram_tensor("w1o", [P, 2 * HIDDEN], dt.float8e4,
                           kind="ExternalInput").ap()
    w2_d = nc.dram_tensor("w2", [P, 8 * D], dt.bfloat16,
                          kind="ExternalInput").ap()
    ident_d = nc.dram_tensor("ident", [P, P], dt.bfloat16,
                             kind="ExternalInput").ap()
    p1si_d = nc.dram_tensor("p1si", [P, 2 * NT * P // 16], dt.int16,
                            kind="ExternalInput").ap()
    p1oi_d = nc.dram_tensor("p1oi", [P, 2 * NT * P // 16], dt.int16,
                            kind="ExternalInput").ap()
    p2i_d = nc.dram_tensor("p2i", [P, 2 * NWIN * NSLOT // 16], dt.int16,
                           kind="ExternalInput").ap()
    oh_d = nc.dram_tensor("oh", [P, 2 * NWIN * NCH * P], dt.float8e4,
                          kind="ExternalInput").ap()
    xw_d = nc.dram_tensor("xw", [2 * HPAD, D], dt.float32,
                          kind="ExternalInput").ap()
    rc_d = nc.dram_tensor("rc", [P, 2 * NWIN], dt.float32,
                          kind="ExternalInput").ap()
    outp = nc.dram_tensor("out", [2 * HPAD, D], dt.float32,
                          kind="ExternalOutput").ap()
    hbuf = nc.dram_tensor("hbuf", [2 * P * NT, HIDDEN], dt.float8e4).ap()
    hbuf3 = hbuf.rearrange("(s p t) h -> s p t h", s=2, p=P)

    with tile.TileContext(nc) as tc:
        with (
            tc.tile_pool(name="const", bufs=1) as const,
            tc.tile_pool(name="gseg", bufs=3) as gseg,
            tc.tile_pool(name="hstg", bufs=2) as hstg,
            tc.tile_pool(name="hgat", bufs=3) as hgat,
            tc.tile_pool(name="ohp", bufs=2) as ohp,
            tc.tile_pool(name="scp", bufs=2) as scp,
            tc.tile_pool(name="stc", bufs=2) as stc,
            tc.tile_pool(name="epi", bufs=2) as epi,
            tc.tile_pool(name="hps", bufs=2, space="PSUM") as hps,
            tc.tile_pool(name="sps", bufs=2, space="PSUM") as sps,
            tc.tile_pool(name="tps", bufs=1, space="PSUM") as tps,
            tc.tile_pool(name="nps", bufs=1, space="PSUM") as nps,
        ):
            # ---- constants
            objb_s = const.tile([P, NODE_RANKS * D], dt.float8e4)
            nc.sync.dma_start(objb_s[:], objb_d[:])
            w1s_s = const.tile([P, 2 * HIDDEN], dt.float8e4)
            nc.sync.dma_start(w1s_s[:], w1s_d[:])
            w1o_s = const.tile([P, 2 * HIDDEN], dt.float8e4)
            nc.sync.dma_start(w1o_s[:], w1o_d[:])
            w2_s = const.tile([P, 8 * D], dt.bfloat16)
            nc.sync.dma_start(w2_s[:], w2_d[:])
            ident_s = const.tile([P, P], dt.bfloat16)
            nc.sync.dma_start(ident_s[:], ident_d[:])
            p1si_s = const.tile([P, 2 * NT * P // 16], dt.int16)
            nc.sync.dma_start(p1si_s[:], p1si_d[:])
            p1oi_s = const.tile([P, 2 * NT * P // 16], dt.int16)
            nc.sync.dma_start(p1oi_s[:], p1oi_d[:])
            p2i_s = const.tile([P, 2 * NWIN * NSLOT // 16], dt.int16)
            nc.sync.dma_start(p2i_s[:], p2i_d[:])
            rc_s = const.tile([P, 2 * NWIN], dt.float32)
            nc.sync.dma_start(rc_s[:], rc_d[:])
            w1sv = w1s_s[:].rearrange("p (two n) -> p two n", two=2)
            w1ov = w1o_s[:].rearrange("p (two n) -> p two n", two=2)
            hi_off = LOSPLIT // P * D            # rank offset (bytes=elems fp8)
            tc.strict_bb_all_engine_barrier()

            # ================= P1 =================
            def p1_half(s):
                for (col0, n_e, cl) in seg_grid:
                    sub_hi, obj_hi = cl >= 2, (cl % 2) == 1
                    ft = gseg.tile([P, 2 * SEG], dt.float8e4, tag="fts")
                    ot = gseg.tile([P, 2 * SEG], dt.float8e4, tag="fto")
                    hstage = hstg.tile([P, (SEG // P) * HIDDEN], dt.float8e4,
                                       tag="hst")
                    for (buf, idx_s, hi) in (
                        (ft, p1si_s, sub_hi), (ot, p1oi_s, obj_hi),
                    ):
                        src = objb_s[:, hi_off:] if hi else objb_s[:]
                        i0 = (s * NT * P + col0) // 16
                        nc.gpsimd.dma_gather(
                            buf[:, : 2 * n_e].rearrange(
                                "p (two m) -> p two m", two=2),
                            src, idx_s[:, i0: i0 + n_e // 16],
                            num_idxs=n_e, num_idxs_reg=n_e,
                            elem_size=D, transpose=True,
                            sbuf_tokens_per_rank=P,
                            sbuf_free_dim_per_rank=D,
                            single_packet=False,
                        )
                    for t in range(n_e // P):
                        hp = hps.tile([P, HIDDEN], dt.float32, tag="hp")
                        nc.tensor.matmul(
                            out=hp[:], lhsT=ft[:, t * 2 * P:(t + 1) * 2 * P],
                            rhs=w1sv, start=True, stop=False,
                            perf_mode=mybir.MatmulPerfMode.DoubleRowSwInterleave,
                        )
                        nc.tensor.matmul(
                            out=hp[:], lhsT=ot[:, t * 2 * P:(t + 1) * 2 * P],
                            rhs=w1ov, start=False, stop=True,
                            perf_mode=mybir.MatmulPerfMode.DoubleRowSwInterleave,
                        )
                        if t % 3 == 0:
                            nc.vector.tensor_scalar_max(
                                hstage[:, t * HIDDEN:(t + 1) * HIDDEN],
                                hp[:], 0.0)
                        else:
                            nc.scalar.activation(
                                out=hstage[:, t * HIDDEN:(t + 1) * HIDDEN],
                                in_=hp[:],
                                func=mybir.ActivationFunctionType.Relu,
                            )
                    t0 = col0 // P
                    nc.sync.dma_start(
                        hbuf3[s, :, t0: t0 + n_e // P, :],
                        hstage[:, : (n_e // P) * HIDDEN].rearrange(
                            "p (t h) -> p t h", h=HIDDEN),
                    )

            # ================= P2/P3 =================
            def p23_half(s):
                hsrc = hbuf[s * P * NT: (s + 1) * P * NT, :]
                for w in range(NWIN):
                    gw = s * NWIN + w
                    hg = hgat.tile([P, NCH * HIDDEN], dt.float8e4, tag="hg")
                    i0 = gw * NSLOT // 16
                    nc.gpsimd.dma_gather(
                        hg[:].rearrange("p (b e) -> p b e", b=NCH),
                        hsrc, p2i_s[:, i0: i0 + NSLOT // 16],
                        num_idxs=NSLOT, num_idxs_reg=NSLOT,
                        elem_size=HIDDEN, elem_step=HIDDEN,
                        single_packet=False,
                    )
                    oh_t = ohp.tile([P, NCH * P], dt.float8e4, tag="oh")
                    nc.scalar.dma_start(
                        oh_t[:], oh_d[:, gw * NCH * P: (gw + 1) * NCH * P])

                    s_sub = sps.tile([P, HIDDEN], dt.float32, tag="ssub")
                    s_obj = sps.tile([P, HIDDEN], dt.float32, tag="sobj")
                    for k in range(NCH):
                        tgt, kk, n_k = (
                            (s_sub, k, CS) if k < CS else (s_obj, k - CS, CO)
                        )
                        nc.tensor.matmul(
                            out=tgt[:],
                            lhsT=oh_t[:, k * P:(k + 1) * P],
                            rhs=hg[:, k * HIDDEN:(k + 1) * HIDDEN],
                            start=(kk == 0), stop=(kk == n_k - 1),
                        )
                    # P3
                    s_sb = scp.tile([P, HIDDEN], dt.bfloat16, tag="ssb")
                    nc.vector.tensor_copy(s_sb[:], s_sub[:])
                    o_sb = scp.tile([P, HIDDEN], dt.bfloat16, tag="osb")
                    nc.vector.tensor_copy(o_sb[:], s_obj[:])
                    stp = tps.tile([P, 8 * P], dt.bfloat16, tag="stp")
                    for b in range(4):
                        nc.tensor.transpose(
                            out=stp[:, b * P:(b + 1) * P],
                            in_=s_sb[:, b * P:(b + 1) * P],
                            identity=ident_s[:])
                    for b in range(4):
                        nc.tensor.transpose(
                            out=stp[:, (4 + b) * P:(5 + b) * P],
                            in_=o_sb[:, b * P:(b + 1) * P],
                            identity=ident_s[:])
                    st_sb = stc.tile([P, 8 * P], dt.bfloat16, tag="stsb")
                    nc.vector.tensor_copy(st_sb[:], stp[:])
                    nmr = nps.tile([P, D], dt.float32, tag="nmr")
                    for b in range(8):
                        nc.tensor.matmul(
                            out=nmr[:], lhsT=st_sb[:, b * P:(b + 1) * P],
                            rhs=w2_s[:, b * D:(b + 1) * D],
                            start=(b == 0), stop=(b == 7),
                        )
                    xw_t = epi.tile([P, D], dt.float32, tag="xwt")
                    nc.scalar.dma_start(
                        xw_t[:], xw_d[s * HPAD + w * P: s * HPAD + (w + 1) * P, :])
                    t1 = epi.tile([P, D], dt.float32, tag="t1")
                    nc.vector.tensor_tensor(
                        out=t1[:], in0=nmr[:], in1=xw_t[:],
                        op=mybir.AluOpType.add)
                    ov = epi.tile([P, D], dt.float32, tag="ov")
                    nc.vector.tensor_scalar_mul(
                        ov[:], t1[:], rc_s[:, gw: gw + 1])
                    nc.sync.dma_start(
                        outp[s * HPAD + w * P: s * HPAD + (w + 1) * P, :], ov[:])

            p1_half(0)
            p1_half(1)
            tc.strict_bb_all_engine_barrier()
            p23_half(0)
            p23_half(1)

    nc.compile()
    return nc


# ================================================================ entry point
def kernel(object_feats, pairs, confidence, W1, b1, W2, b2):
    in_maps, dims = _preprocess(object_feats, pairs, confidence, W1, b1, W2, b2)
    if dims not in _BUILD_CACHE:
        _BUILD_CACHE[dims] = _build_program(dims)
    nc = _BUILD_CACHE[dims]
    res = run_bass_kernel_spmd(nc, in_maps, core_ids=list(range(N_CORES)))
    out = np.empty((O_NODES, D), dtype=np.float32)
    for c in range(N_CORES):
        o = res.results[c]["out"]
        out[2 * c::N_HALF] = o[:HSHARD]
        out[2 * c + 1::N_HALF] = o[HPAD:HPAD + HSHARD]
    return out.astype(np.float32)
